# revision 1
# baseline (speedup 1.0000x reference)
"""Additive attention (Bahdanau-style) on 8 TRN2 NeuronCores.

Reference computation (S=1024, B=64, QK=H=DV=1024):
    q = queries @ W_q.T          [S,B,H]
    k = keys    @ W_k.T          [S,B,H]
    f = tanh(q + k)              [S,B,H]
    scores = f @ w_v.T           [S,B,1]
    attn = softmax(scores, axis=S)
    out[b,d] = sum_s attn[s,b] * values[s,0,d]    -> [B,DV]

Strategy: pure data parallel over B (8 batches per core), weights/values
replicated, no collectives.  Per core the dominant work is one fused
matmul [S*BL, 2*QK] @ [2*QK, H] (queries/keys concatenated along the
contraction dim), computed in bf16 with the contraction dim
pre-transposed onto SBUF partitions host-side.  z is produced in
[h, sb] layout (weights stationary); tanh runs on the scalar engine out
of PSUM; scores are produced directly transposed ([sb,1] columns) by
f-stationary matmuls against the w_v column, so exp'd scores land in
the [s, b] layout the final attn^T @ V matmul needs with no shuffle.
Softmax normalization folds into the output copy as a per-partition
1/den scale.

This walrus build rejects engine instructions with more than one
embedded sync wait, so the program must be built as bacc.Bacc and run
through Bacc.compile(): its generate_event_semaphores pass splits
excess on_wait entries onto standalone InstEventSemaphore carriers.
(The optional act_observe machinery below predates that discovery and
is off by default.)

Measured on TRN2: ~488-492 us HW exec (8 cores SPMD), vs a ~464 us
per-core PE-busy floor (446 us z-matmul stream at the bf16 1-cycle/row
rate + ~18 us score/output matmuls) plus ~27 us fixed preamble, DMA
pipeline fill, and drain.  The PE stream runs with <2 us of total gaps;
prologue experiments (earlier PE start, parallel trigger dispatch,
finer first-chunk DMAs) all converged to the same total because the
ramp is DMA-pipeline-fill bound.  Note the chip clock is bimodal under
sustained benchmarking: ~2.4 GHz rested vs ~2.0 GHz hot (+20% wall).
"""

import numpy as np
import ml_dtypes

P = 128
CORES = 8
CHUNK_W = 512   # token-chunk width; build_nc and prep_in_maps must agree

FULL_S, FULL_B, FULL_QK, FULL_H, FULL_DV = 1024, 64, 1024, 1024, 1024


def build_nc(S=FULL_S, BL=FULL_B // CORES, QK2=2 * FULL_QK, H=FULL_H,
             DV=FULL_DV, CW=CHUNK_W, XT_BUFS=4, Z_BUFS=3, use_observers=False):
    """Build the single-core Bacc program (same NEFF runs SPMD on all cores)."""
    import concourse.bass as bass
    import concourse.mybir as mybir
    import concourse.tile as tile
    from concourse import bacc

    dt = mybir.dt
    f32, bf16 = dt.float32, dt.bfloat16
    AF = mybir.ActivationFunctionType

    SB = S * BL          # tokens, b-major: sb = b*S + s
    KO = QK2 // P        # contraction subtiles
    HT = H // P          # h tiles
    CH = SB // CW        # token chunks (each chunk: one b, CW s-values)
    SBLK = S // P        # s blocks (final matmul contraction tiles)
    OCW = min(CW, DV)    # output free-dim chunk
    DT = DV // OCW
    TPC = CW // P        # transposed score sub-blocks per chunk
    KQ = max(1, KO // 4)  # xt DMA split granularity
    assert SB % CW == 0 and QK2 % P == 0 and H % P == 0 and S % P == 0
    assert S % CW == 0 and CW % P == 0

    nc = bacc.Bacc("TRN2", debug=False, target_bir_lowering=False)

    # chunk-major host layouts: each SBUF partition reads one contiguous
    # slab per DMA (minimal descriptor count, full queue bandwidth)
    xt = nc.dram_tensor("xt", [CH, P, KO, CW], bf16, kind="ExternalInput").ap()
    wct = nc.dram_tensor("wct", [P, KO, H], bf16, kind="ExternalInput").ap()
    # wv columns per h-tile, plus a trailing all-ones column
    wv = nc.dram_tensor("wv", [P, HT + 1], bf16, kind="ExternalInput").ap()
    vals = nc.dram_tensor("vals", [P, SBLK, DV], bf16, kind="ExternalInput").ap()
    out = nc.dram_tensor("out", [BL, DV], f32, kind="ExternalOutput").ap()

    with tile.TileContext(nc) as tc:
        with tc.tile_pool(name="const", bufs=1) as const_pool, \
             tc.tile_pool(name="xtp", bufs=XT_BUFS) as xt_pool, \
             tc.tile_pool(name="fp", bufs=2 * HT + 2) as f_pool, \
             tc.tile_pool(name="obs", bufs=2) as obs_pool, \
             tc.tile_pool(name="misc", bufs=2) as misc_pool, \
             tc.tile_pool(name="zps", bufs=Z_BUFS, space="PSUM") as zpsum, \
             tc.tile_pool(name="tps", bufs=2, space="PSUM") as tpsum, \
             tc.tile_pool(name="dps", bufs=1, space="PSUM") as dpsum, \
             tc.tile_pool(name="ops", bufs=2, space="PSUM") as opsum:

            # first xt chunk starts streaming before everything else so
            # the PE can begin the z stream as early as possible
            # separate tiles per ko slice / xt quarter so each z matmul
            # depends only on the one DMA that feeds it (whole-tile dep
            # tracking would otherwise gate the first matmul on ALL loads)
            NQ = (KO + KQ - 1) // KQ
            xt_tile0 = [xt_pool.tile([P, KQ, CW], bf16, tag=f"xtq{j}",
                                     name=f"xt0_q{j}") for j in range(NQ)]
            wct_ks = [const_pool.tile([P, H], bf16, name=f"wct_k{ko}")
                      for ko in range(KO)]
            wv_sb = const_pool.tile([P, HT + 1], bf16)
            for j in range(NQ):
                nc.sync.dma_start(xt_tile0[j][:],
                                  xt[0, :, j * KQ:(j + 1) * KQ, :])
                if j == 0:
                    nc.sync.dma_start(wct_ks[0][:], wct[:, 0, :])
                    nc.sync.dma_start(wv_sb[:], wv[:])
            for ko in range(1, KO):
                nc.sync.dma_start(wct_ks[ko][:], wct[:, ko, :])
            vals_sb = const_pool.tile([P, SBLK, DV], bf16)
            ones_col = wv_sb[:, HT:HT + 1]
            # exp(scores) in [s partitions, sblk, b] layout
            a_sb = const_pool.tile([P, SBLK, BL], bf16)

            # ACT-clock observer: a 1-element copy whose only dep is the
            # previous ACT output, so the following activation needs just
            # its PE wait.
            last_act = [None]

            def act_observe():
                if use_observers and last_act[0] is not None:
                    o = obs_pool.tile([1, 1], f32, tag="obs")
                    nc.scalar.activation(o[:], last_act[0], AF.Copy)

            SH = S // CW
            den_psum = dpsum.tile([BL, 1], f32)
            o_psums = [opsum.tile([BL, OCW], f32, tag="o", name=f"o_psum{d}") for d in range(DT)]

            def final_mms(sblk_range):
                for i, sblk in enumerate(sblk_range):
                    nc.tensor.matmul(
                        den_psum[:], lhsT=a_sb[:, sblk, :], rhs=ones_col,
                        start=(sblk == 0), stop=(sblk == SBLK - 1),
                        skip_group_check=True)
                for d in range(DT):
                    for sblk in sblk_range:
                        nc.tensor.matmul(
                            o_psums[d][:], lhsT=a_sb[:, sblk, :],
                            rhs=vals_sb[:, sblk, d * OCW:(d + 1) * OCW],
                            start=(sblk == 0), stop=(sblk == SBLK - 1),
                            skip_group_check=True)

            prev = None

            def emit_scores(f_tiles, shalf, b):
                for t in range(TPC):
                    tr_psum = tpsum.tile([P, 1], f32, tag="tr")
                    for h in range(HT):
                        nc.tensor.matmul(
                            tr_psum[:],
                            lhsT=f_tiles[h][:, t * P:(t + 1) * P],
                            rhs=wv_sb[:, h:h + 1],
                            start=(h == 0), stop=(h == HT - 1),
                            skip_group_check=True)
                    sblk = shalf * TPC + t
                    act_observe()
                    nc.scalar.activation(a_sb[:, sblk, b:b + 1], tr_psum[:],
                                         AF.Exp)
                    last_act[0] = a_sb[0:1, sblk, b:b + 1]

            for c in range(CH):
                shalf = c // BL
                b = c % BL
                if c == 0:
                    xt_tile = xt_tile0
                else:
                    cc = b * SH + shalf
                    xt_tile = [xt_pool.tile([P, KQ, CW], bf16, tag=f"xtq{j}",
                                            name=f"xt{c}_q{j}")
                               for j in range(NQ)]
                    for j in range(NQ):
                        nc.sync.dma_start(xt_tile[j][:],
                                          xt[cc, :, j * KQ:(j + 1) * KQ, :])
                if c == 2:
                    nc.sync.dma_start(vals_sb[:], vals[:])
                f_tiles = []
                for h in range(HT):
                    z_psum = zpsum.tile([P, CW], f32)
                    for ko in range(KO):
                        nc.tensor.matmul(
                            z_psum[:],
                            lhsT=wct_ks[ko][:, h * P:(h + 1) * P],
                            rhs=xt_tile[ko // KQ][:, ko % KQ, :],
                            start=(ko == 0), stop=(ko == KO - 1))
                    f_tile = f_pool.tile([P, CW], bf16)
                    act_observe()
                    nc.scalar.activation(f_tile[:], z_psum[:], AF.Tanh)
                    last_act[0] = f_tile[0:1, 0:1]
                    f_tiles.append(f_tile)
                # score phase pipelined one chunk behind the z stream so
                # the in-order PE never stalls on the current chunk's last
                # tanh (head-of-line blocking at chunk boundaries)
                if prev is not None:
                    emit_scores(*prev)
                prev = (f_tiles, shalf, b)
                if c > 0 and c % BL == 0:
                    # scores for all chunks of the previous s-half are now
                    # emitted: run that slice of the output matmuls
                    final_mms(range((shalf - 1) * TPC, shalf * TPC))

            emit_scores(*prev)
            final_mms(range((SH - 1) * TPC, SH * TPC))
            den_inv = misc_pool.tile([BL, 1], f32, tag="dinv")
            nc.vector.reciprocal(den_inv[:], den_psum[:])
            # let ACT observe the DVE tick so the scaled output copies
            # carry only their PE wait
            obs_d = obs_pool.tile([1, 1], f32, tag="obs")
            nc.scalar.activation(obs_d[:], den_inv[0:1, 0:1], AF.Copy)

            for d in range(DT):
                o_psum = o_psums[d]
                o_sb = misc_pool.tile([BL, OCW], f32, tag=f"o{d}")
                act_observe()
                nc.scalar.activation(o_sb[:], o_psum[:], AF.Copy,
                                     scale=den_inv[:])
                last_act[0] = o_sb[0:1, 0:1]
                nc.sync.dma_start(out[:, d * OCW:(d + 1) * OCW], o_sb[:])
    return nc


def prep_in_maps(queries, keys, values, W_q, W_k, w_v, n_cores=CORES):
    """Host-side shard + transpose (b-major) + bf16 cast."""
    bf = ml_dtypes.bfloat16
    queries = np.asarray(queries, dtype=np.float32)
    keys = np.asarray(keys, dtype=np.float32)
    S, B, QK = queries.shape
    BL = B // n_cores
    H = np.asarray(W_q).shape[0]
    HT = H // P

    q_bf = queries.astype(bf)
    k_bf = keys.astype(bf)
    KO = 2 * QK // P
    wct_np = np.ascontiguousarray(
        np.concatenate([np.asarray(W_q, np.float32),
                        np.asarray(W_k, np.float32)], axis=1).T
        .astype(bf).reshape(KO, P, H).transpose(1, 0, 2))
    wv_np = np.empty((P, HT + 1), dtype=bf)
    wv_np[:, :HT] = np.asarray(w_v, np.float32).reshape(HT, P).T.astype(bf)
    wv_np[:, HT] = np.float32(1.0)
    DV = np.asarray(values).shape[2]
    vals_np = np.ascontiguousarray(
        np.asarray(values, np.float32)[:, 0, :].astype(bf)
        .reshape(S // P, P, DV).transpose(1, 0, 2))

    in_maps = []
    for c in range(n_cores):
        # [S, BL, QK] -> [QK, BL, S] -> [QK, BL*S]   (sb = b*S + s)
        qT = np.ascontiguousarray(
            q_bf[:, c * BL:(c + 1) * BL, :].transpose(2, 1, 0)).reshape(QK, S * BL)
        kT = np.ascontiguousarray(
            k_bf[:, c * BL:(c + 1) * BL, :].transpose(2, 1, 0)).reshape(QK, S * BL)
        xt_2d = np.concatenate([qT, kT], axis=0)       # [2QK, BL*S]
        CW = CHUNK_W
        CH = S * BL // CW
        xt_np = np.ascontiguousarray(
            xt_2d.reshape(KO, P, CH, CW).transpose(2, 1, 0, 3))
        in_maps.append({"xt": xt_np, "wct": wct_np, "wv": wv_np,
                        "vals": vals_np})
    return in_maps


_NC_CACHE = {}


def _get_nc():
    if "nc" not in _NC_CACHE:
        nc = build_nc()
        nc.finalize()
        _NC_CACHE["nc"] = nc
    return _NC_CACHE["nc"]


def kernel_with_results(trace=False, **inputs):
    from concourse.bass_utils import run_bass_kernel_spmd
    nc = _get_nc()
    in_maps = prep_in_maps(**inputs)
    res = run_bass_kernel_spmd(nc, in_maps, core_ids=list(range(CORES)),
                               trace=trace)
    out = np.concatenate([np.asarray(res.results[i]["out"], np.float32)
                          for i in range(CORES)], axis=0)
    return out, res


def kernel(**inputs):
    out, _ = kernel_with_results(trace=False, **inputs)
    return out



# revision 5
# speedup vs baseline: 1.0670x; 1.0670x over previous
"""Additive attention on 8 TRN2 cores — fp8 DoubleRow z-stream variant.

Same structure as the bf16 baseline (chunk-major z stream, scores
pipelined one chunk behind, softmax norm folded into the output copy),
but the z contraction runs mostly in fp8e4 with perf_mode=DoubleRow
(2 fp8 weights per PE cell -> 2x MACs/cycle).  NB of the 16 k-blocks
stay bf16.  The fp8 quantization error is mostly cancelled by a
host-computed per-token bias riding the exp() activation's bias port:

    score_est[t] = sum_h wv_h tanh(zhat_h[t]) + c*(u.x[t] - u_eff.x_eff[t])

with u = wv@W exact, u_eff.x_eff the same rank-1 functional on the
quantized operands, c ~ E[tanh'(z)] (fit: 0.48).  Numpy sim rel err:
NB=0 -> 1.88e-2, NB=4 -> 1.63e-2 (tolerance 2e-2, bf16 baseline 3.5e-3).

fp8 scaling: x*16, W*32 before the e4m3 cast (normal range, max ~90/5
vs limit 240); tanh activation scale=1/512 undoes it; bf16 blend
operands carry the same exact power-of-2 scales.
"""

import numpy as np
import ml_dtypes

P = 128
CORES = 8
CHUNK_W = 512
NB = 4            # k-blocks (of 16) kept in bf16; must keep (16-NB)%4==0
C_FIX = 0.48
SX, SW = 16.0, 32.0

FULL_S, FULL_B, FULL_QK, FULL_H, FULL_DV = 1024, 64, 1024, 1024, 1024


def build_nc(S=FULL_S, BL=FULL_B // CORES, QK2=2 * FULL_QK, H=FULL_H,
             DV=FULL_DV, CW=CHUNK_W, nb=NB, XT_BUFS=4, Z_BUFS=3):
    import concourse.mybir as mybir
    import concourse.tile as tile
    from concourse import bacc

    dt = mybir.dt
    f32, bf16, f8 = dt.float32, dt.bfloat16, dt.float8e4
    AF = mybir.ActivationFunctionType
    PM = mybir.MatmulPerfMode

    SB = S * BL
    KO = QK2 // P        # 16 k-blocks
    KF = KO - nb         # fp8 k-blocks
    KP = KF // 2         # DoubleRow pairs
    HT = H // P
    CH = SB // CW
    SBLK = S // P
    OCW = min(CW, DV)
    DT = DV // OCW
    TPC = CW // P
    SH = S // CW
    NQ8 = KF // 4        # xt8 tiles per chunk (4 blocks = 2 pairs each)
    assert KF % 4 == 0 and SB % CW == 0 and S % CW == 0

    nc = bacc.Bacc("TRN2", debug=False, target_bir_lowering=False)

    xt8 = nc.dram_tensor("xt8", [CH, P, KF, CW], f8, kind="ExternalInput").ap()
    wct8 = nc.dram_tensor("wct8", [P, KF, H], f8, kind="ExternalInput").ap()
    if nb:
        xtb = nc.dram_tensor("xtb", [CH, P, nb, CW], bf16,
                             kind="ExternalInput").ap()
        wctb = nc.dram_tensor("wctb", [P, nb, H], bf16,
                              kind="ExternalInput").ap()
    wv = nc.dram_tensor("wv", [P, HT + 1], bf16, kind="ExternalInput").ap()
    vals = nc.dram_tensor("vals", [P, SBLK, DV], bf16, kind="ExternalInput").ap()
    bias = nc.dram_tensor("bias", [P, SBLK, BL], f32, kind="ExternalInput").ap()
    out = nc.dram_tensor("out", [BL, DV], f32, kind="ExternalOutput").ap()

    with tile.TileContext(nc) as tc:
        with tc.tile_pool(name="const", bufs=1) as const_pool, \
             tc.tile_pool(name="xtp", bufs=XT_BUFS) as xt_pool, \
             tc.tile_pool(name="fp", bufs=2 * HT + 2) as f_pool, \
             tc.tile_pool(name="misc", bufs=2) as misc_pool, \
             tc.tile_pool(name="zps", bufs=Z_BUFS, space="PSUM") as zpsum, \
             tc.tile_pool(name="tps", bufs=2, space="PSUM") as tpsum, \
             tc.tile_pool(name="dps", bufs=1, space="PSUM") as dpsum, \
             tc.tile_pool(name="ops", bufs=2, space="PSUM") as opsum:

            def load_chunk(c, stop_after=None):
                """DMA one chunk's moving data.  stop_after=j pauses after
                fp8 tile j so startup can interleave weight DMAs."""
                b, shalf = c % BL, c // BL
                cc = b * SH + shalf
                t8s = []
                for j in range(NQ8):
                    t = xt_pool.tile([P, 4, CW], f8, tag=f"xt8q{j}",
                                     name=f"xt8_{c}_q{j}")
                    nc.sync.dma_start(t[:], xt8[cc, :, 4 * j:4 * j + 4, :])
                    t8s.append(t)
                    if stop_after == j:
                        return t8s, None
                tb = None
                if nb:
                    tb = xt_pool.tile([P, nb, CW], bf16, tag="xtb",
                                      name=f"xtb_{c}")
                    nc.sync.dma_start(tb[:], xtb[cc, :, :, :])
                return t8s, tb

            KPt = KF // 2
            wct8_ps = [const_pool.tile([P, 2, H], f8, name=f"wct8_p{kp}")
                       for kp in range(KPt)]
            wctb_ks = [const_pool.tile([P, H], bf16, name=f"wctb_k{kb}")
                       for kb in range(nb)]
            wv_sb = const_pool.tile([P, HT + 1], bf16)
            vals_sb = const_pool.tile([P, SBLK, DV], bf16)
            bias_sb = const_pool.tile([P, SBLK, BL], f32)
            a_sb = const_pool.tile([P, SBLK, BL], bf16)
            ones_col = wv_sb[:, HT:HT + 1]

            # startup order: first fp8 third of chunk 0, then ALL z-stream
            # weights (the h-loop cycles through every pair within ~1.5us
            # of the first matmul), then the rest of chunk 0
            t8s0, _ = load_chunk(0, stop_after=0)
            for kp in range(KPt):
                nc.sync.dma_start(wct8_ps[kp][:], wct8[:, 2 * kp:2 * kp + 2, :])
            for j in range(1, NQ8):
                t = xt_pool.tile([P, 4, CW], f8, tag=f"xt8q{j}",
                                 name=f"xt8_0_q{j}")
                nc.sync.dma_start(t[:], xt8[0, :, 4 * j:4 * j + 4, :])
                t8s0.append(t)
            for kb in range(nb):
                nc.sync.dma_start(wctb_ks[kb][:], wctb[:, kb, :])
            tb0 = None
            if nb:
                tb0 = xt_pool.tile([P, nb, CW], bf16, tag="xtb",
                                   name="xtb_0")
                nc.sync.dma_start(tb0[:], xtb[0, :, :, :])
            tiles = {0: (t8s0, tb0)}
            nc.sync.dma_start(wv_sb[:], wv[:])
            nc.sync.dma_start(bias_sb[:], bias[:])

            den_psum = dpsum.tile([BL, 1], f32)
            o_psums = [opsum.tile([BL, OCW], f32, tag="o", name=f"o_psum{d}")
                       for d in range(DT)]

            def final_mms(sblk_range):
                for sblk in sblk_range:
                    nc.tensor.matmul(
                        den_psum[:], lhsT=a_sb[:, sblk, :], rhs=ones_col,
                        start=(sblk == 0), stop=(sblk == SBLK - 1),
                        skip_group_check=True)
                for d in range(DT):
                    for sblk in sblk_range:
                        nc.tensor.matmul(
                            o_psums[d][:], lhsT=a_sb[:, sblk, :],
                            rhs=vals_sb[:, sblk, d * OCW:(d + 1) * OCW],
                            start=(sblk == 0), stop=(sblk == SBLK - 1),
                            skip_group_check=True)

            def emit_scores(f_tiles, shalf, b):
                for t in range(TPC):
                    tr_psum = tpsum.tile([P, 1], f32, tag="tr")
                    for h in range(HT):
                        nc.tensor.matmul(
                            tr_psum[:],
                            lhsT=f_tiles[h][:, t * P:(t + 1) * P],
                            rhs=wv_sb[:, h:h + 1],
                            start=(h == 0), stop=(h == HT - 1),
                            skip_group_check=True)
                    sblk = shalf * TPC + t
                    nc.scalar.activation(a_sb[:, sblk, b:b + 1], tr_psum[:],
                                         AF.Exp,
                                         bias=bias_sb[:, sblk, b:b + 1])

            prev = None
            for c in range(CH):
                shalf = c // BL
                b = c % BL
                if c + 1 < CH:
                    tiles[c + 1] = load_chunk(c + 1)
                if c == 2:
                    nc.sync.dma_start(vals_sb[:], vals[:])
                t8s, tb = tiles.pop(c)
                f_tiles = []
                pend = None   # z_psum awaiting blend+tanh (lag 1 behind DR)

                def finish_h(z_psum, h):
                    for kb in range(nb):
                        nc.tensor.matmul(
                            z_psum[:],
                            lhsT=wctb_ks[kb][:, h * P:(h + 1) * P],
                            rhs=tb[:, kb, :],
                            start=False, stop=(kb == nb - 1),
                            skip_group_check=True)
                    f_tile = f_pool.tile([P, CW], bf16, tag="f")
                    nc.scalar.activation(f_tile[:], z_psum[:], AF.Tanh,
                                         scale=1.0 / (SX * SW))
                    f_tiles.append(f_tile)

                for h in range(HT):
                    z_psum = zpsum.tile([P, CW], f32)
                    for kp in range(KP):
                        nc.tensor.matmul(
                            z_psum[:],
                            lhsT=wct8_ps[kp][:, :, h * P:(h + 1) * P],
                            rhs=t8s[kp // 2][:, 2 * (kp % 2):2 * (kp % 2) + 2, :],
                            start=(kp == 0), stop=(nb == 0 and kp == KP - 1),
                            perf_mode=PM.DoubleRow,
                            skip_group_check=True)
                    if nb == 0:
                        finish_h(z_psum, h)
                    else:
                        if pend is not None:
                            finish_h(*pend)
                        pend = (z_psum, h)
                if pend is not None:
                    finish_h(*pend)
                if prev is not None:
                    emit_scores(*prev)
                prev = (f_tiles, shalf, b)
                if c > 0 and c % BL == 0:
                    final_mms(range((shalf - 1) * TPC, shalf * TPC))

            emit_scores(*prev)
            final_mms(range((SH - 1) * TPC, SH * TPC))

            den_inv = misc_pool.tile([BL, 1], f32, tag="dinv")
            nc.vector.reciprocal(den_inv[:], den_psum[:])
            for d in range(DT):
                o_sb = misc_pool.tile([BL, OCW], f32, tag=f"o{d}")
                nc.scalar.activation(o_sb[:], o_psums[d][:], AF.Copy,
                                     scale=den_inv[:])
                nc.sync.dma_start(out[:, d * OCW:(d + 1) * OCW], o_sb[:])
    return nc


def prep_in_maps(queries, keys, values, W_q, W_k, w_v, n_cores=CORES,
                 nb=NB, c_fix=C_FIX):
    bf = ml_dtypes.bfloat16
    e4 = ml_dtypes.float8_e4m3
    queries = np.asarray(queries, dtype=np.float32)
    keys = np.asarray(keys, dtype=np.float32)
    S, B, QK = queries.shape
    BL = B // n_cores
    H = np.asarray(W_q).shape[0]
    HT = H // P
    KO = 2 * QK // P
    KF = KO - nb
    CW = CHUNK_W
    CH = S * BL // CW
    SBLK = S // P
    nbc = nb * P          # bf16 contraction columns (they go LAST)

    Wcat = np.concatenate([np.asarray(W_q, np.float32),
                           np.asarray(W_k, np.float32)], axis=1)  # [H, 2QK]
    # fp8 blocks first, bf16 blend blocks last
    W8 = (Wcat[:, :KF * P] * SW).astype(e4)
    Wb = (Wcat[:, KF * P:] * SW).astype(bf) if nb else None
    wct8_np = np.ascontiguousarray(
        W8.T.reshape(KF, P, H).transpose(1, 0, 2))
    wctb_np = (np.ascontiguousarray(
        Wb.T.reshape(nb, P, H).transpose(1, 0, 2)) if nb else None)

    wvb = np.asarray(w_v, np.float32).reshape(H)
    wv_np = np.empty((P, HT + 1), dtype=bf)
    wv_np[:, :HT] = wvb.reshape(HT, P).T.astype(bf)
    wv_np[:, HT] = np.float32(1.0)
    wvb32 = wv_np[:, :HT].astype(np.float32).T.reshape(H)

    DV = np.asarray(values).shape[2]
    vals_np = np.ascontiguousarray(
        np.asarray(values, np.float32)[:, 0, :].astype(bf)
        .reshape(S // P, P, DV).transpose(1, 0, 2))

    u_exact = wvb32 @ Wcat                                  # [2QK]
    u_8 = (wvb32 @ W8.astype(np.float32)) / SW
    u_b = (wvb32 @ Wb.astype(np.float32)) / SW if nb else None

    in_maps = []
    for cidx in range(n_cores):
        q = queries[:, cidx * BL:(cidx + 1) * BL, :]
        k = keys[:, cidx * BL:(cidx + 1) * BL, :]
        qT = np.ascontiguousarray(q.transpose(2, 1, 0)).reshape(QK, S * BL)
        kT = np.ascontiguousarray(k.transpose(2, 1, 0)).reshape(QK, S * BL)
        xt2d = np.concatenate([qT, kT], axis=0)             # [2QK, SB] f32
        x8 = (xt2d[:KF * P] * SX).astype(e4)
        xb = (xt2d[KF * P:] * SX).astype(bf) if nb else None
        xt8_np = np.ascontiguousarray(
            x8.reshape(KF, P, CH, CW).transpose(2, 1, 0, 3))
        xtb_np = (np.ascontiguousarray(
            xb.reshape(nb, P, CH, CW).transpose(2, 1, 0, 3)) if nb else None)

        dot_exact = u_exact @ xt2d
        dot_eff = u_8 @ (x8.astype(np.float32) / SX)
        if nb:
            dot_eff = dot_eff + u_b @ (xb.astype(np.float32) / SX)
        r = c_fix * (dot_exact - dot_eff)
        bias_np = np.ascontiguousarray(
            r.reshape(BL, SBLK, P).transpose(2, 1, 0)).astype(np.float32)

        m = {"xt8": xt8_np, "wct8": wct8_np, "wv": wv_np,
             "vals": vals_np, "bias": bias_np}
        if nb:
            m["xtb"] = xtb_np
            m["wctb"] = wctb_np
        in_maps.append(m)
    return in_maps


_NC_CACHE = {}


def _get_nc():
    if "nc" not in _NC_CACHE:
        nc = build_nc()
        nc.finalize()
        _NC_CACHE["nc"] = nc
    return _NC_CACHE["nc"]


def kernel_with_results(trace=False, **inputs):
    from concourse.bass_utils import run_bass_kernel_spmd
    nc = _get_nc()
    in_maps = prep_in_maps(**inputs)
    res = run_bass_kernel_spmd(nc, in_maps, core_ids=list(range(CORES)),
                               trace=trace)
    out = np.concatenate([np.asarray(res.results[i]["out"], np.float32)
                          for i in range(CORES)], axis=0)
    return out, res


def kernel(**inputs):
    out, _ = kernel_with_results(trace=False, **inputs)
    return out


# revision 7
# speedup vs baseline: 1.0839x; 1.0159x over previous
"""Additive attention on 8 TRN2 cores — fp8 DoubleRow + top-|wv| bf16 h-tiles.

z-stream structure (per 512-token chunk, h permuted by |wv_h| descending
on the host so tile 0 holds the largest-|wv| 128 h's):
  - h-tiles HB..7: x8 @ W8 in fp8e4 perf_mode=DoubleRow (2 k-blocks per
    512-cycle matmul -> 2x MACs)  [8 DR matmuls each]
  - h-tiles 0..HB-1: exact bf16 x @ W                [16 matmuls each]
Score error is sum_h wv_h^2-weighted, and the top 128 |wv_h| carry 49%
of sum wv^2 (top 256: 71%), so spending bf16 exactness there buys far
more accuracy per cycle than spreading it across k-blocks.

The remaining fp8 quantization error is mostly cancelled by a
host-computed per-token bias riding the exp() activation bias port:
    score_est[t] = sum_h wv_h tanh(zhat_h[t]) + c*(u.x[t] - u_eff.x_eff[t])
with u = wv@W exact, u_eff.x_eff the same rank-1 functional on the
quantized operands the device uses, c ~ E[tanh'(z)] = 0.48.

Numpy sim rel err (same inputs as the harness): HB=1 -> 1.34e-2,
HB=2 -> 1.04e-2 (tolerance 2e-2; all-bf16 baseline 3.5e-3).

fp8 scaling: x*16, W*32 pre-cast (normal range, max ~90/5 vs limit
240); tanh scale=1/512 undoes it; bf16 operands carry the same exact
power-of-2 scales.
"""

import numpy as np
import ml_dtypes

P = 128
CORES = 8
CHUNK_W = 512
HB = 1            # top-|wv| h-tiles computed in bf16
C_FIX = 0.48
SX, SW = 16.0, 32.0

FULL_S, FULL_B, FULL_QK, FULL_H, FULL_DV = 1024, 64, 1024, 1024, 1024


def build_nc(S=FULL_S, BL=FULL_B // CORES, QK2=2 * FULL_QK, H=FULL_H,
             DV=FULL_DV, CW=CHUNK_W, hb=HB, XT_BUFS=4, Z_BUFS=3):
    import concourse.mybir as mybir
    import concourse.tile as tile
    from concourse import bacc

    dt = mybir.dt
    f32, bf16, f8 = dt.float32, dt.bfloat16, dt.float8e4
    AF = mybir.ActivationFunctionType
    PM = mybir.MatmulPerfMode

    SB = S * BL
    KO = QK2 // P        # 16 k-blocks
    KP = KO // 2         # 8 DoubleRow pairs
    HT = H // P          # 8 h-tiles; tiles 0..hb-1 are bf16
    HBC = hb * P
    H8 = H - HBC         # fp8 h columns
    CH = SB // CW
    SBLK = S // P
    OCW = min(CW, DV)
    DT = DV // OCW
    TPC = CW // P
    SH = S // CW
    NQ8 = KO // 4        # xt8 tiles per chunk (4 blocks = 2 pairs each)
    NQB = 2              # xbf tiles per chunk (8 blocks each)
    assert SB % CW == 0 and S % CW == 0 and 1 <= hb < HT

    nc = bacc.Bacc("TRN2", debug=False, target_bir_lowering=False)

    xt8 = nc.dram_tensor("xt8", [CH, P, KO, CW], f8, kind="ExternalInput").ap()
    xbf = nc.dram_tensor("xbf", [CH, P, KO, CW], bf16, kind="ExternalInput").ap()
    wct8 = nc.dram_tensor("wct8", [P, KO, H8], f8, kind="ExternalInput").ap()
    wcb = nc.dram_tensor("wcb", [P, KO, HBC], bf16, kind="ExternalInput").ap()
    wv = nc.dram_tensor("wv", [P, HT + 1], bf16, kind="ExternalInput").ap()
    vals = nc.dram_tensor("vals", [P, SBLK, DV], bf16, kind="ExternalInput").ap()
    bias = nc.dram_tensor("bias", [P, SBLK, BL], f32, kind="ExternalInput").ap()
    out = nc.dram_tensor("out", [BL, DV], f32, kind="ExternalOutput").ap()

    with tile.TileContext(nc) as tc:
        with tc.tile_pool(name="const", bufs=1) as const_pool, \
             tc.tile_pool(name="xtp", bufs=XT_BUFS) as xt_pool, \
             tc.tile_pool(name="fp", bufs=2 * HT + 2) as f_pool, \
             tc.tile_pool(name="misc", bufs=2) as misc_pool, \
             tc.tile_pool(name="zps", bufs=Z_BUFS, space="PSUM") as zpsum, \
             tc.tile_pool(name="tps", bufs=2, space="PSUM") as tpsum, \
             tc.tile_pool(name="dps", bufs=1, space="PSUM") as dpsum, \
             tc.tile_pool(name="ops", bufs=2, space="PSUM") as opsum:

            wct8_ps = [const_pool.tile([P, 2, H8], f8, name=f"wct8_p{kp}")
                       for kp in range(KP)]
            wcb_ks = [const_pool.tile([P, HBC], bf16, name=f"wcb_k{ko}")
                      for ko in range(KO)]
            wv_sb = const_pool.tile([P, HT + 1], bf16)
            vals_sb = const_pool.tile([P, SBLK, DV], bf16)
            bias_sb = const_pool.tile([P, SBLK, BL], f32)
            a_sb = const_pool.tile([P, SBLK, BL], bf16)
            ones_col = wv_sb[:, HT:HT + 1]

            def load_chunk(c, t8_only=False):
                b, shalf = c % BL, c // BL
                cc = b * SH + shalf
                t8s = []
                for j in range(NQ8):
                    t = xt_pool.tile([P, 4, CW], f8, tag=f"xt8q{j}",
                                     name=f"xt8_{c}_q{j}")
                    nc.sync.dma_start(t[:], xt8[cc, :, 4 * j:4 * j + 4, :])
                    t8s.append(t)
                if t8_only:
                    return t8s, None
                tbs = []
                for j in range(NQB):
                    KH = KO // NQB
                    t = xt_pool.tile([P, KH, CW], bf16, tag=f"xbfq{j}",
                                     name=f"xbf_{c}_q{j}")
                    nc.sync.dma_start(t[:], xbf[cc, :, KH * j:KH * (j + 1), :])
                    tbs.append(t)
                return t8s, tbs

            # startup: chunk0 fp8 stream, then all fp8 weights (first
            # matmuls cycle through every pair within ~2us), then the
            # bf16 side, then chunk0 bf16 stream
            t8s0, _ = load_chunk(0, t8_only=True)
            for kp in range(KP):
                nc.sync.dma_start(wct8_ps[kp][:], wct8[:, 2 * kp:2 * kp + 2, :])
            for ko in range(KO):
                nc.sync.dma_start(wcb_ks[ko][:], wcb[:, ko, :])
            tbs0 = []
            for j in range(NQB):
                KH = KO // NQB
                t = xt_pool.tile([P, KH, CW], bf16, tag=f"xbfq{j}",
                                 name=f"xbf_0_q{j}")
                nc.sync.dma_start(t[:], xbf[0, :, KH * j:KH * (j + 1), :])
                tbs0.append(t)
            tiles = {0: (t8s0, tbs0)}
            nc.sync.dma_start(wv_sb[:], wv[:])
            nc.sync.dma_start(bias_sb[:], bias[:])

            den_psum = dpsum.tile([BL, 1], f32)
            o_psums = [opsum.tile([BL, OCW], f32, tag="o", name=f"o_psum{d}")
                       for d in range(DT)]

            def final_mms(sblk_range):
                for sblk in sblk_range:
                    nc.tensor.matmul(
                        den_psum[:], lhsT=a_sb[:, sblk, :], rhs=ones_col,
                        start=(sblk == 0), stop=(sblk == SBLK - 1),
                        skip_group_check=True)
                for d in range(DT):
                    for sblk in sblk_range:
                        nc.tensor.matmul(
                            o_psums[d][:], lhsT=a_sb[:, sblk, :],
                            rhs=vals_sb[:, sblk, d * OCW:(d + 1) * OCW],
                            start=(sblk == 0), stop=(sblk == SBLK - 1),
                            skip_group_check=True)

            def emit_scores(f_tiles, shalf, b):
                for t in range(TPC):
                    tr_psum = tpsum.tile([P, 1], f32, tag="tr")
                    for h in range(HT):
                        nc.tensor.matmul(
                            tr_psum[:],
                            lhsT=f_tiles[h][:, t * P:(t + 1) * P],
                            rhs=wv_sb[:, h:h + 1],
                            start=(h == 0), stop=(h == HT - 1),
                            skip_group_check=True)
                    sblk = shalf * TPC + t
                    nc.scalar.activation(a_sb[:, sblk, b:b + 1], tr_psum[:],
                                         AF.Exp,
                                         bias=bias_sb[:, sblk, b:b + 1])

            prev = None
            for c in range(CH):
                shalf = c // BL
                b = c % BL
                if c + 1 < CH:
                    tiles[c + 1] = load_chunk(c + 1)
                if c == 2:
                    nc.sync.dma_start(vals_sb[:], vals[:])
                t8s, tbs = tiles.pop(c)
                f_tiles = [None] * HT

                def tanh_out(z_psum, h):
                    f_tile = f_pool.tile([P, CW], bf16, tag="f")
                    nc.scalar.activation(f_tile[:], z_psum[:], AF.Tanh,
                                         scale=1.0 / (SX * SW))
                    f_tiles[h] = f_tile

                # fp8 tiles first (their data/weights arrive first)
                for h in range(hb, HT):
                    hc = (h - hb) * P
                    z_psum = zpsum.tile([P, CW], f32, tag="z",
                                        name=f"z_{c}_{h}")
                    for kp in range(KP):
                        nc.tensor.matmul(
                            z_psum[:],
                            lhsT=wct8_ps[kp][:, :, hc:hc + P],
                            rhs=t8s[kp // 2][:, 2 * (kp % 2):2 * (kp % 2) + 2, :],
                            start=(kp == 0), stop=(kp == KP - 1),
                            perf_mode=PM.DoubleRow,
                            skip_group_check=True)
                    tanh_out(z_psum, h)
                # bf16 top tiles last
                KH = KO // NQB
                for h in range(hb):
                    z_psum = zpsum.tile([P, CW], f32, tag="z",
                                        name=f"zb_{c}_{h}")
                    for ko in range(KO):
                        nc.tensor.matmul(
                            z_psum[:],
                            lhsT=wcb_ks[ko][:, h * P:(h + 1) * P],
                            rhs=tbs[ko // KH][:, ko % KH, :],
                            start=(ko == 0), stop=(ko == KO - 1),
                            skip_group_check=True)
                    tanh_out(z_psum, h)

                if prev is not None:
                    emit_scores(*prev)
                prev = (f_tiles, shalf, b)
                # one chunk later than strictly needed so the a_sb slices
                # are certainly written and the PE never stalls here
                if c % BL == 1 and c > 1:
                    final_mms(range((shalf - 1) * TPC, shalf * TPC))

            emit_scores(*prev)
            final_mms(range((SH - 1) * TPC, SH * TPC))

            den_inv = misc_pool.tile([BL, 1], f32, tag="dinv")
            nc.vector.reciprocal(den_inv[:], den_psum[:])
            for d in range(DT):
                o_sb = misc_pool.tile([BL, OCW], f32, tag=f"o{d}")
                nc.scalar.activation(o_sb[:], o_psums[d][:], AF.Copy,
                                     scale=den_inv[:])
                nc.sync.dma_start(out[:, d * OCW:(d + 1) * OCW], o_sb[:])
    return nc


def prep_in_maps(queries, keys, values, W_q, W_k, w_v, n_cores=CORES,
                 hb=HB, c_fix=C_FIX):
    bf = ml_dtypes.bfloat16
    e4 = ml_dtypes.float8_e4m3
    queries = np.asarray(queries, dtype=np.float32)
    keys = np.asarray(keys, dtype=np.float32)
    S, B, QK = queries.shape
    BL = B // n_cores
    H = np.asarray(W_q).shape[0]
    HT = H // P
    KO = 2 * QK // P
    CW = CHUNK_W
    CH = S * BL // CW
    SBLK = S // P
    HBC = hb * P

    wvb_raw = np.asarray(w_v, np.float32).reshape(H)
    wvb0 = wvb_raw.astype(bf).astype(np.float32)
    perm = np.argsort(-np.abs(wvb0), kind="stable")

    Wcat = np.concatenate([np.asarray(W_q, np.float32),
                           np.asarray(W_k, np.float32)], axis=1)[perm]  # [H,2QK]
    wvp = wvb0[perm]

    Wb = (Wcat[:HBC] * SW).astype(bf)                     # [HBC, 2QK]
    W8 = (Wcat[HBC:] * SW).astype(e4)                     # [H-HBC, 2QK]
    wcb_np = np.ascontiguousarray(
        Wb.T.reshape(KO, P, HBC).transpose(1, 0, 2))
    wct8_np = np.ascontiguousarray(
        W8.T.reshape(KO, P, H - HBC).transpose(1, 0, 2))

    wv_np = np.empty((P, HT + 1), dtype=bf)
    wv_np[:, :HT] = wvp.reshape(HT, P).T.astype(bf)
    wv_np[:, HT] = np.float32(1.0)

    DV = np.asarray(values).shape[2]
    vals_np = np.ascontiguousarray(
        np.asarray(values, np.float32)[:, 0, :].astype(bf)
        .reshape(S // P, P, DV).transpose(1, 0, 2))

    u_exact = wvp @ Wcat                                  # [2QK]
    u_b = (wvp[:HBC] @ Wb.astype(np.float32)) / SW
    u_8 = (wvp[HBC:] @ W8.astype(np.float32)) / SW

    in_maps = []
    for cidx in range(n_cores):
        q = queries[:, cidx * BL:(cidx + 1) * BL, :]
        k = keys[:, cidx * BL:(cidx + 1) * BL, :]
        qT = np.ascontiguousarray(q.transpose(2, 1, 0)).reshape(QK, S * BL)
        kT = np.ascontiguousarray(k.transpose(2, 1, 0)).reshape(QK, S * BL)
        xt2d = np.concatenate([qT, kT], axis=0)           # [2QK, SB] f32
        x8 = (xt2d * SX).astype(e4)
        xb = (xt2d * SX).astype(bf)
        xt8_np = np.ascontiguousarray(
            x8.reshape(KO, P, CH, CW).transpose(2, 1, 0, 3))
        xbf_np = np.ascontiguousarray(
            xb.reshape(KO, P, CH, CW).transpose(2, 1, 0, 3))

        dot_exact = u_exact @ xt2d
        dot_eff = (u_b @ (xb.astype(np.float32) / SX)
                   + u_8 @ (x8.astype(np.float32) / SX))
        r = c_fix * (dot_exact - dot_eff)
        bias_np = np.ascontiguousarray(
            r.reshape(BL, SBLK, P).transpose(2, 1, 0)).astype(np.float32)

        in_maps.append({"xt8": xt8_np, "xbf": xbf_np, "wct8": wct8_np,
                        "wcb": wcb_np, "wv": wv_np, "vals": vals_np,
                        "bias": bias_np})
    return in_maps


_NC_CACHE = {}


def _get_nc():
    if "nc" not in _NC_CACHE:
        nc = build_nc()
        nc.finalize()
        _NC_CACHE["nc"] = nc
    return _NC_CACHE["nc"]


def kernel_with_results(trace=False, **inputs):
    from concourse.bass_utils import run_bass_kernel_spmd
    nc = _get_nc()
    in_maps = prep_in_maps(**inputs)
    res = run_bass_kernel_spmd(nc, in_maps, core_ids=list(range(CORES)),
                               trace=trace)
    out = np.concatenate([np.asarray(res.results[i]["out"], np.float32)
                          for i in range(CORES)], axis=0)
    return out, res


def kernel(**inputs):
    out, _ = kernel_with_results(trace=False, **inputs)
    return out


# revision 8
# speedup vs baseline: 1.0987x; 1.0136x over previous
"""Additive attention on 8 TRN2 cores — fp8 DoubleRow + top-|wv| bf16 h-tiles.

z-stream structure (per 512-token chunk, h permuted by |wv_h| descending
on the host so tile 0 holds the largest-|wv| 128 h's):
  - h-tiles HB..7: x8 @ W8 in fp8e4 perf_mode=DoubleRow (2 k-blocks per
    512-cycle matmul -> 2x MACs)  [8 DR matmuls each]
  - h-tiles 0..HB-1: exact bf16 x @ W                [16 matmuls each]
Score error is sum_h wv_h^2-weighted, and the top 128 |wv_h| carry 49%
of sum wv^2 (top 256: 71%), so spending bf16 exactness there buys far
more accuracy per cycle than spreading it across k-blocks.

The remaining fp8 quantization error is mostly cancelled by a
host-computed per-token bias riding the exp() activation bias port:
    score_est[t] = sum_h wv_h tanh(zhat_h[t]) + c*(u.x[t] - u_eff.x_eff[t])
with u = wv@W exact, u_eff.x_eff the same rank-1 functional on the
quantized operands the device uses, c ~ E[tanh'(z)] = 0.48.

Numpy sim rel err (same inputs as the harness): HB=1 -> 1.34e-2,
HB=2 -> 1.04e-2 (tolerance 2e-2; all-bf16 baseline 3.5e-3).
Measured on TRN2 (8 cores SPMD): 296.8us HW exec, rel err 1.3387e-2
(sim and HW agree to ~0.1% relative on the error statistic), vs the
all-bf16 baseline's 488us / 3.5e-3.  PE stream is ~93% of the span:
z 1152 matmuls x 216ns = 249us + scores ~22us; preamble ~15us.

fp8 scaling: x*16, W*32 pre-cast (normal range, max ~90/5 vs limit
240); tanh scale=1/512 undoes it; bf16 operands carry the same exact
power-of-2 scales.
"""

import numpy as np
import ml_dtypes

P = 128
CORES = 8
CHUNK_W = 512
HB = 1            # top-|wv| h-tiles computed in bf16
C_FIX = 0.48
SX, SW = 16.0, 32.0

FULL_S, FULL_B, FULL_QK, FULL_H, FULL_DV = 1024, 64, 1024, 1024, 1024


def build_nc(S=FULL_S, BL=FULL_B // CORES, QK2=2 * FULL_QK, H=FULL_H,
             DV=FULL_DV, CW=CHUNK_W, hb=HB, XT_BUFS=4, Z_BUFS=3):
    import concourse.mybir as mybir
    import concourse.tile as tile
    from concourse import bacc

    dt = mybir.dt
    f32, bf16, f8 = dt.float32, dt.bfloat16, dt.float8e4
    AF = mybir.ActivationFunctionType
    PM = mybir.MatmulPerfMode

    SB = S * BL
    KO = QK2 // P        # 16 k-blocks
    KP = KO // 2         # 8 DoubleRow pairs
    HT = H // P          # 8 h-tiles; tiles 0..hb-1 are bf16
    HBC = hb * P
    H8 = H - HBC         # fp8 h columns
    CH = SB // CW
    SBLK = S // P
    OCW = min(CW, DV)
    DT = DV // OCW
    TPC = CW // P
    SH = S // CW
    NQ8 = KO // 4        # xt8 tiles per chunk (4 blocks = 2 pairs each)
    NQB = 2              # xbf tiles per chunk (8 blocks each)
    assert SB % CW == 0 and S % CW == 0 and 1 <= hb < HT

    nc = bacc.Bacc("TRN2", debug=False, target_bir_lowering=False)

    xt8 = nc.dram_tensor("xt8", [CH, P, KO, CW], f8, kind="ExternalInput").ap()
    xbf = nc.dram_tensor("xbf", [CH, P, KO, CW], bf16, kind="ExternalInput").ap()
    wct8 = nc.dram_tensor("wct8", [P, KO, H8], f8, kind="ExternalInput").ap()
    wcb = nc.dram_tensor("wcb", [P, KO, HBC], bf16, kind="ExternalInput").ap()
    wv = nc.dram_tensor("wv", [P, HT + 1], bf16, kind="ExternalInput").ap()
    vals = nc.dram_tensor("vals", [P, SBLK, DV], bf16, kind="ExternalInput").ap()
    bias = nc.dram_tensor("bias", [P, SBLK, BL], f32, kind="ExternalInput").ap()
    out = nc.dram_tensor("out", [BL, DV], f32, kind="ExternalOutput").ap()

    with tile.TileContext(nc) as tc:
        with tc.tile_pool(name="const", bufs=1) as const_pool, \
             tc.tile_pool(name="xtp", bufs=XT_BUFS) as xt_pool, \
             tc.tile_pool(name="fp", bufs=2 * HT + 2) as f_pool, \
             tc.tile_pool(name="misc", bufs=2) as misc_pool, \
             tc.tile_pool(name="zps", bufs=Z_BUFS, space="PSUM") as zpsum, \
             tc.tile_pool(name="tps", bufs=2, space="PSUM") as tpsum, \
             tc.tile_pool(name="dps", bufs=1, space="PSUM") as dpsum, \
             tc.tile_pool(name="ops", bufs=2, space="PSUM") as opsum:

            wct8_ps = [const_pool.tile([P, 2, H8], f8, name=f"wct8_p{kp}")
                       for kp in range(KP)]
            wcb_ks = [const_pool.tile([P, HBC], bf16, name=f"wcb_k{ko}")
                      for ko in range(KO)]
            wv_sb = const_pool.tile([P, HT + 1], bf16)
            vals_sb = const_pool.tile([P, SBLK, DV], bf16)
            bias_sb = const_pool.tile([P, SBLK, BL], f32)
            a_sb = const_pool.tile([P, SBLK, BL], bf16)
            ones_col = wv_sb[:, HT:HT + 1]

            def load_chunk(c, t8_only=False):
                b, shalf = c % BL, c // BL
                cc = b * SH + shalf
                t8s = []
                for j in range(NQ8):
                    t = xt_pool.tile([P, 4, CW], f8, tag=f"xt8q{j}",
                                     name=f"xt8_{c}_q{j}")
                    nc.sync.dma_start(t[:], xt8[cc, :, 4 * j:4 * j + 4, :])
                    t8s.append(t)
                if t8_only:
                    return t8s, None
                tbs = []
                for j in range(NQB):
                    KH = KO // NQB
                    t = xt_pool.tile([P, KH, CW], bf16, tag=f"xbfq{j}",
                                     name=f"xbf_{c}_q{j}")
                    nc.sync.dma_start(t[:], xbf[cc, :, KH * j:KH * (j + 1), :])
                    tbs.append(t)
                return t8s, tbs

            # startup: chunk0 fp8 stream, then all fp8 weights (first
            # matmuls cycle through every pair within ~2us), then the
            # bf16 side, then chunk0 bf16 stream
            t8s0, _ = load_chunk(0, t8_only=True)
            for kp in range(KP):
                nc.sync.dma_start(wct8_ps[kp][:], wct8[:, 2 * kp:2 * kp + 2, :])
            for ko in range(KO):
                nc.sync.dma_start(wcb_ks[ko][:], wcb[:, ko, :])
            tbs0 = []
            for j in range(NQB):
                KH = KO // NQB
                t = xt_pool.tile([P, KH, CW], bf16, tag=f"xbfq{j}",
                                 name=f"xbf_0_q{j}")
                nc.sync.dma_start(t[:], xbf[0, :, KH * j:KH * (j + 1), :])
                tbs0.append(t)
            tiles = {0: (t8s0, tbs0)}
            nc.sync.dma_start(wv_sb[:], wv[:])
            nc.sync.dma_start(bias_sb[:], bias[:])

            den_psum = dpsum.tile([BL, 1], f32)
            o_psums = [opsum.tile([BL, OCW], f32, tag="o", name=f"o_psum{d}")
                       for d in range(DT)]

            def final_mms(sblk_range):
                for sblk in sblk_range:
                    nc.tensor.matmul(
                        den_psum[:], lhsT=a_sb[:, sblk, :], rhs=ones_col,
                        start=(sblk == 0), stop=(sblk == SBLK - 1),
                        skip_group_check=True)
                for d in range(DT):
                    for sblk in sblk_range:
                        nc.tensor.matmul(
                            o_psums[d][:], lhsT=a_sb[:, sblk, :],
                            rhs=vals_sb[:, sblk, d * OCW:(d + 1) * OCW],
                            start=(sblk == 0), stop=(sblk == SBLK - 1),
                            skip_group_check=True)

            def emit_scores(f_tiles, shalf, b):
                for t in range(TPC):
                    tr_psum = tpsum.tile([P, 1], f32, tag="tr")
                    for h in range(HT):
                        nc.tensor.matmul(
                            tr_psum[:],
                            lhsT=f_tiles[h][:, t * P:(t + 1) * P],
                            rhs=wv_sb[:, h:h + 1],
                            start=(h == 0), stop=(h == HT - 1),
                            skip_group_check=True)
                    sblk = shalf * TPC + t
                    nc.scalar.activation(a_sb[:, sblk, b:b + 1], tr_psum[:],
                                         AF.Exp,
                                         bias=bias_sb[:, sblk, b:b + 1])

            prev = None
            for c in range(CH):
                shalf = c // BL
                b = c % BL
                if c + 1 < CH:
                    tiles[c + 1] = load_chunk(c + 1)
                if c == 2:
                    nc.sync.dma_start(vals_sb[:], vals[:])
                t8s, tbs = tiles.pop(c)
                f_tiles = [None] * HT

                def tanh_out(z_psum, h):
                    f_tile = f_pool.tile([P, CW], bf16, tag="f")
                    nc.scalar.activation(f_tile[:], z_psum[:], AF.Tanh,
                                         scale=1.0 / (SX * SW))
                    f_tiles[h] = f_tile

                # fp8 tiles first (their data/weights arrive first)
                for h in range(hb, HT):
                    hc = (h - hb) * P
                    z_psum = zpsum.tile([P, CW], f32, tag="z",
                                        name=f"z_{c}_{h}")
                    for kp in range(KP):
                        nc.tensor.matmul(
                            z_psum[:],
                            lhsT=wct8_ps[kp][:, :, hc:hc + P],
                            rhs=t8s[kp // 2][:, 2 * (kp % 2):2 * (kp % 2) + 2, :],
                            start=(kp == 0), stop=(kp == KP - 1),
                            perf_mode=PM.DoubleRow,
                            skip_group_check=True)
                    tanh_out(z_psum, h)
                # bf16 top tiles last
                KH = KO // NQB
                for h in range(hb):
                    z_psum = zpsum.tile([P, CW], f32, tag="z",
                                        name=f"zb_{c}_{h}")
                    for ko in range(KO):
                        nc.tensor.matmul(
                            z_psum[:],
                            lhsT=wcb_ks[ko][:, h * P:(h + 1) * P],
                            rhs=tbs[ko // KH][:, ko % KH, :],
                            start=(ko == 0), stop=(ko == KO - 1),
                            skip_group_check=True)
                    tanh_out(z_psum, h)

                if prev is not None:
                    emit_scores(*prev)
                prev = (f_tiles, shalf, b)
                # one chunk later than strictly needed so the a_sb slices
                # are certainly written and the PE never stalls here
                if c % BL == 1 and c > 1:
                    final_mms(range((shalf - 1) * TPC, shalf * TPC))

            emit_scores(*prev)
            final_mms(range((SH - 1) * TPC, SH * TPC))

            den_inv = misc_pool.tile([BL, 1], f32, tag="dinv")
            nc.vector.reciprocal(den_inv[:], den_psum[:])
            for d in range(DT):
                o_sb = misc_pool.tile([BL, OCW], f32, tag=f"o{d}")
                nc.scalar.activation(o_sb[:], o_psums[d][:], AF.Copy,
                                     scale=den_inv[:])
                nc.sync.dma_start(out[:, d * OCW:(d + 1) * OCW], o_sb[:])
    return nc


def prep_in_maps(queries, keys, values, W_q, W_k, w_v, n_cores=CORES,
                 hb=HB, c_fix=C_FIX):
    bf = ml_dtypes.bfloat16
    e4 = ml_dtypes.float8_e4m3
    queries = np.asarray(queries, dtype=np.float32)
    keys = np.asarray(keys, dtype=np.float32)
    S, B, QK = queries.shape
    BL = B // n_cores
    H = np.asarray(W_q).shape[0]
    HT = H // P
    KO = 2 * QK // P
    CW = CHUNK_W
    CH = S * BL // CW
    SBLK = S // P
    HBC = hb * P

    wvb_raw = np.asarray(w_v, np.float32).reshape(H)
    wvb0 = wvb_raw.astype(bf).astype(np.float32)
    perm = np.argsort(-np.abs(wvb0), kind="stable")

    Wcat = np.concatenate([np.asarray(W_q, np.float32),
                           np.asarray(W_k, np.float32)], axis=1)[perm]  # [H,2QK]
    wvp = wvb0[perm]

    Wb = (Wcat[:HBC] * SW).astype(bf)                     # [HBC, 2QK]
    W8 = (Wcat[HBC:] * SW).astype(e4)                     # [H-HBC, 2QK]
    wcb_np = np.ascontiguousarray(
        Wb.T.reshape(KO, P, HBC).transpose(1, 0, 2))
    wct8_np = np.ascontiguousarray(
        W8.T.reshape(KO, P, H - HBC).transpose(1, 0, 2))

    wv_np = np.empty((P, HT + 1), dtype=bf)
    wv_np[:, :HT] = wvp.reshape(HT, P).T.astype(bf)
    wv_np[:, HT] = np.float32(1.0)

    DV = np.asarray(values).shape[2]
    vals_np = np.ascontiguousarray(
        np.asarray(values, np.float32)[:, 0, :].astype(bf)
        .reshape(S // P, P, DV).transpose(1, 0, 2))

    u_exact = wvp @ Wcat                                  # [2QK]
    u_b = (wvp[:HBC] @ Wb.astype(np.float32)) / SW
    u_8 = (wvp[HBC:] @ W8.astype(np.float32)) / SW

    in_maps = []
    for cidx in range(n_cores):
        q = queries[:, cidx * BL:(cidx + 1) * BL, :]
        k = keys[:, cidx * BL:(cidx + 1) * BL, :]
        qT = np.ascontiguousarray(q.transpose(2, 1, 0)).reshape(QK, S * BL)
        kT = np.ascontiguousarray(k.transpose(2, 1, 0)).reshape(QK, S * BL)
        xt2d = np.concatenate([qT, kT], axis=0)           # [2QK, SB] f32
        x8 = (xt2d * SX).astype(e4)
        xb = (xt2d * SX).astype(bf)
        xt8_np = np.ascontiguousarray(
            x8.reshape(KO, P, CH, CW).transpose(2, 1, 0, 3))
        xbf_np = np.ascontiguousarray(
            xb.reshape(KO, P, CH, CW).transpose(2, 1, 0, 3))

        dot_exact = u_exact @ xt2d
        dot_eff = (u_b @ (xb.astype(np.float32) / SX)
                   + u_8 @ (x8.astype(np.float32) / SX))
        r = c_fix * (dot_exact - dot_eff)
        bias_np = np.ascontiguousarray(
            r.reshape(BL, SBLK, P).transpose(2, 1, 0)).astype(np.float32)

        in_maps.append({"xt8": xt8_np, "xbf": xbf_np, "wct8": wct8_np,
                        "wcb": wcb_np, "wv": wv_np, "vals": vals_np,
                        "bias": bias_np})
    return in_maps


_NC_CACHE = {}


def _get_nc():
    if "nc" not in _NC_CACHE:
        nc = build_nc()
        nc.finalize()
        _NC_CACHE["nc"] = nc
    return _NC_CACHE["nc"]


def kernel_with_results(trace=False, **inputs):
    from concourse.bass_utils import run_bass_kernel_spmd
    nc = _get_nc()
    in_maps = prep_in_maps(**inputs)
    res = run_bass_kernel_spmd(nc, in_maps, core_ids=list(range(CORES)),
                               trace=trace)
    out = np.concatenate([np.asarray(res.results[i]["out"], np.float32)
                          for i in range(CORES)], axis=0)
    return out, res


def kernel(**inputs):
    out, _ = kernel_with_results(trace=False, **inputs)
    return out
